# revision 1
# baseline (speedup 1.0000x reference)
"""Trainium2 Bass kernel for 16-head MHA (E=1024, S=2048, B=4) on 8 NeuronCores.

Sharding: tensor-parallel over head groups (TP=2: heads 0-7 / 8-15) x
data-parallel over batch (DP=4).  Core c handles batch c//2, head group c%2.
Each core computes its 8 heads end-to-end plus the out-projection restricted
to its heads' rows of W_out; the host sums the two TP partials and adds b_out.

Device-side dataflow per core (matmuls in float32r at full PE rate):
  phase V : V[s, h*64+d] = x @ Wv + bv          (bias via K=1 augmented matmul)
  phase QK: QK^T[m, s]   = [Wq*0.125 | Wk]^T chunks @ x^T   (bias fused in the
            PSUM->SBUF copy; attention scale pre-folded into Wq/bq on host)
  attn    : scores^T[t, s] per head (2 heads packed in the 128-wide PE array
            via tile_position), exp on ACT over 1024-wide PSUM APs, then
            o_aug[65, s] = [V | 1]^T @ exp_scores^T giving o_unnorm^T and the
            softmax denominator in one accumulation; normalize with
            reciprocal + gpsimd partition_broadcast + DVE multiply.
  out     : out[s, e] = sum_k O^T[k-chunk, s-tile]^T @ Wo[k-chunk, e]
"""

import numpy as np

import concourse.bass as bass
import concourse.tile as tile
from concourse import bacc, mybir
from concourse.alu_op_type import AluOpType
from concourse.bass_utils import run_bass_kernel_spmd

F32 = mybir.dt.float32
F32R = mybir.dt.float32r
MM_DT = F32R      # matmul operand dtype: F32R (fast) or F32 (exact, 4x slower)
EXP = mybir.ActivationFunctionType.Exp

E = 1024          # embed dim
S = 2048          # sequence
B = 4             # batch
NH = 16           # total heads
HD = 64           # head dim
TP = 2            # head-group shards
HPC = NH // TP    # heads per core = 8
QKW = HPC * HD * 2   # 1024 q+k columns per core
VW = HPC * HD        # 512 v columns per core

KCH = E // 128      # 8 contraction chunks
MT = QKW // 128     # 8 qk^T row tiles (0-3 Q, 4-7 K)
ST = S // 128       # 16 sequence tiles
SC = S // 512       # 4 sequence 512-chunks

_CACHE = {}


def _mm(ap):
    """DRAM-side view matching the MM operand dtype (byte-identical)."""
    return ap.bitcast(MM_DT) if MM_DT is not F32 else ap


def build_nc():
    nc = bacc.Bacc("TRN2", target_bir_lowering=False, debug=False, num_devices=8)

    xT_d = nc.dram_tensor("xT", [E, S], F32, kind="ExternalInput").ap()
    wqk_d = nc.dram_tensor("wqk", [E, QKW], F32, kind="ExternalInput").ap()
    bqk_d = nc.dram_tensor("bqk", [128, MT], F32, kind="ExternalInput").ap()
    wv_d = nc.dram_tensor("wv", [E, VW], F32, kind="ExternalInput").ap()
    bv_d = nc.dram_tensor("bv", [1, VW], F32, kind="ExternalInput").ap()
    wo_d = nc.dram_tensor("wo", [VW, E], F32, kind="ExternalInput").ap()
    out_d = nc.dram_tensor("out", [S, E], F32, kind="ExternalOutput").ap()

    xT_t = xT_d.rearrange("(k p) s -> k p s", p=128)

    with tile.TileContext(nc) as tc:
        with (tc.tile_pool(name="persist", bufs=1) as pp,
              tc.tile_pool(name="qkdram", bufs=1, space="DRAM") as qkd,
              tc.tile_pool(name="otp", bufs=1) as otp):
            bqk_sb = pp.tile([128, MT], F32, tag="bqk")
            bv_sb = pp.tile([1, VW], MM_DT, tag="bv")
            ones_f32 = pp.tile([128, 128], F32, tag="ones_f32")
            nc.vector.memset(ones_f32[:], 1.0)
            ones_sb = pp.tile([1, 128], MM_DT, tag="ones")
            nc.vector.tensor_copy(ones_sb[:], ones_f32[0:1, :])

            vaug = [pp.tile([128, HPC * 65], MM_DT, tag=f"va{st}", name=f"va{st}")
                    for st in range(ST)]
            qk_dram = [qkd.tile([128, S], MM_DT, tag=f"qkd{m}", name=f"qkd{m}")
                       for m in range(MT)]
            ot = [otp.tile([128, S], MM_DT, tag=f"ot{hp}", name=f"ot{hp}")
                  for hp in range(TP * 2)]

            with (tc.tile_pool(name="scps", bufs=2, space="PSUM") as scps,
                  tc.tile_pool(name="oaps", bufs=2, space="PSUM") as oaps,
                  tc.tile_pool(name="p1qk", bufs=2) as p1qk,
                  tc.tile_pool(name="attn", bufs=3) as ap_):
                with tc.tile_pool(name="p1x", bufs=1) as p1x:
                    # hp0's QK weight tiles first — they gate the very first
                    # matmuls; wv (needed ~30us in) loads after the x^T chunks
                    def load_wm(m):
                        wm = p1qk.tile([128, E], MM_DT, tag="wm", name=f"wm{m}")
                        nc.sync.dma_start(
                            wm[:].rearrange("p (k c) -> p k c", c=128),
                            _mm(wqk_d[:, m * 128:(m + 1) * 128].rearrange(
                                "(k p) c -> p k c", p=128)))
                        return wm

                    wm4 = load_wm(4)
                    wm0 = load_wm(0)
                    nc.sync.dma_start(bqk_sb[:], bqk_d[:])
                    nc.sync.dma_start(bv_sb[:], _mm(bv_d[:]))
                    xt_all = p1x.tile([128, KCH * S], MM_DT, tag="xt")
                    for k in range(KCH):
                        eng = nc.scalar if k % 2 == 0 else nc.sync
                        eng.dma_start(
                            xt_all[:, k * S:(k + 1) * S], _mm(xT_t[k]))
                    xt = [xt_all[:, k * S:(k + 1) * S] for k in range(KCH)]
                    wv_all = p1qk.tile([128, KCH * VW], MM_DT,
                                       tag="stage", bufs=1)
                    nc.scalar.dma_start(
                        wv_all[:].rearrange("p (k c) -> p k c", c=VW),
                        _mm(wv_d.rearrange("(k p) c -> p k c", p=128)))
                    wv_sb = [wv_all[:, k * VW:(k + 1) * VW] for k in range(KCH)]

                    # ---- QK projection interleaved with attention head-pairs;
                    # ---- biased QK^T m-tiles spill through DRAM scratch.
                    # ---- The V projection shares the same 2 PSUM slots.
                    qkps = tc.alloc_tile_pool(name="qkps", bufs=2, space="PSUM")

                    def emit_qk(m, dest=None, wm=None):
                        # dest=None: spill the biased m-tile to DRAM scratch;
                        # else write straight into the attention SBUF tile
                        if wm is None:
                            wm = load_wm(m)
                        if dest is None:
                            dest = p1qk.tile([128, S], MM_DT, tag="stage",
                                             bufs=1, name=f"stage{m}")
                        for j in range(SC):
                            ps = qkps.tile([128, 512], F32, tag="qk",
                                           name=f"qkps{m}_{j}")
                            for k in range(KCH):
                                nc.tensor.matmul(
                                    ps[:], wm[:, k * 128:(k + 1) * 128],
                                    xt[k][:, j * 512:(j + 1) * 512],
                                    start=(k == 0), stop=(k == KCH - 1))
                            nc.vector.tensor_scalar_add(
                                dest[:, j * 512:(j + 1) * 512], ps[:],
                                bqk_sb[:, m:m + 1])
                        if dest.tensor.name.startswith("stage"):
                            nc.sync.dma_start(qk_dram[m][:], dest[:])

                    # hp0's K/Q projections first so attention can start
                    # as early as possible
                    kt0 = ap_.tile([128, S], MM_DT, tag="ktc", bufs=2)
                    qt0 = ap_.tile([128, S], MM_DT, tag="qtc", bufs=2)
                    emit_qk(4, dest=kt0, wm=wm4)
                    emit_qk(0, dest=qt0, wm=wm0)

                    # ---------- phase V: V + bias in [s, h*64+d] layout ------
                    # emitted AFTER attention hp0 below so hp0's scores/exp get
                    # PE priority; V matmuls fill the exp-bound PE gaps via
                    # backpressure. vaug[st] deps keep the V-matmuls honest.
                    def emit_v_tile(st):
                        vp = qkps.tile([128, VW], F32, tag="qk",
                                       name=f"vp{st}")
                        for k in range(KCH):
                            nc.tensor.matmul(
                                vp[:], xt[k][:, st * 128:(st + 1) * 128],
                                wv_sb[k][:], start=(k == 0), stop=False)
                        nc.tensor.matmul(
                            vp[:], ones_sb[:, :128], bv_sb[:],
                            start=False, stop=True)
                        va = vaug[st].rearrange("p (h c) -> p h c", c=65)
                        nc.vector.tensor_copy(va[:, :, 64:65],
                                              ones_f32[:, 0:8])
                        nc.vector.tensor_copy(
                            va[:, :, 0:64],
                            vp[:].rearrange("p (h d) -> p h d", d=64))

                    for hp in range(HPC // 2):
                        if hp == 0:
                            attention_head_pair(nc, tc, scps, oaps, ap_,
                                                qk_dram, vaug, ot, hp,
                                                kt=kt0, qt=qt0,
                                                pre_vmm=[emit_v_tile]
                                                * 0 or
                                                [lambda s=s: emit_v_tile(s)
                                                 for s in range(ST)])
                        elif hp == 1:
                            kt = ap_.tile([128, S], MM_DT, tag="ktc", bufs=2)
                            qt = ap_.tile([128, S], MM_DT, tag="qtc", bufs=2)
                            emit_qk(4 + hp, dest=kt)
                            emit_qk(hp, dest=qt)
                            attention_head_pair(nc, tc, scps, oaps, ap_,
                                                qk_dram, vaug, ot, hp,
                                                kt=kt, qt=qt)
                        else:
                            emit_qk(4 + hp)
                            emit_qk(hp)
                            attention_head_pair(nc, tc, scps, oaps, ap_,
                                                qk_dram, vaug, ot, hp)
                    qkps.release()

                # ---- phase 3: out projection (overlaps hp3 attention) ----
                ops = tc.alloc_tile_pool(name="ops", bufs=1, space="PSUM")
                with (tc.tile_pool(name="wop", bufs=1) as wop,
                      tc.tile_pool(name="obp", bufs=3) as obp):
                    wo_all = wop.tile([128, 4 * E], MM_DT, tag="wo")
                    nc.sync.dma_start(
                        wo_all[:].rearrange("p (k c) -> p k c", c=E),
                        _mm(wo_d.rearrange("(k p) c -> p k c", p=128)))
                    wo_sb = [wo_all[:, k * E:(k + 1) * E] for k in range(4)]
                    for st in range(ST):
                        ob = obp.tile([128, E], F32, tag="ob")
                        for e in range(2):
                            op = ops.tile([128, 512], F32, tag="op", bufs=2)
                            for k in range(4):
                                nc.tensor.matmul(
                                    op[:],
                                    ot[k][:, st * 128:(st + 1) * 128],
                                    wo_sb[k][:, e * 512:(e + 1) * 512],
                                    start=(k == 0), stop=(k == 3))
                            nc.vector.tensor_copy(
                                ob[:, e * 512:(e + 1) * 512], op[:])
                        nc.sync.dma_start(
                            out_d[st * 128:(st + 1) * 128, :], ob[:])
                ops.release()

    nc.compile()
    return nc


def attention_head_pair(nc, tc, scps, oaps, ap_, qk_dram, vaug, ot, hp,
                        kt=None, qt=None, pre_vmm=None):
    if kt is None:
        kt = ap_.tile([128, S], MM_DT, tag="ktc", bufs=2, name=f"kt{hp}")
        nc.sync.dma_start(kt[:], qk_dram[4 + hp][:])
        qt = ap_.tile([128, S], MM_DT, tag="qtc", bufs=2, name=f"qt{hp}")
        nc.sync.dma_start(qt[:], qk_dram[hp][:])

    def drain(oa0, oa1, j):
        for h, oa in ((0, oa0), (1, oa1)):
            recip = ap_.tile([1, 512], F32, tag="recip", bufs=2)
            nc.vector.reciprocal(recip[:], oa[64:65, :])
            rb = ap_.tile([64, 512], F32, tag="rb", bufs=2)
            nc.gpsimd.partition_broadcast(rb[:], recip[:])
            nc.vector.tensor_tensor(
                ot[hp][h * 64:(h + 1) * 64, j * 512:(j + 1) * 512],
                oa[0:64, :], rb[:], op=AluOpType.mult)

    def vmm(et, oa0, oa1, t, j):
        # V-matmuls for t-tile `t`, one pipeline step behind scores/exp
        for h, oa in ((0, oa0), (1, oa1)):
            hh = hp * 2 + h
            nc.tensor.matmul(
                oa[:], vaug[t][:, hh * 65:(hh + 1) * 65],
                et[:, h * 512:(h + 1) * 512],
                start=(t == 0), stop=(t == ST - 1))
        if t == ST - 1:
            drain(oa0, oa1, j)

    pending = None
    for j in range(SC):
        oa0 = oaps.tile([65, 512], F32, tag="oa", name=f"oa0_{hp}_{j}")
        oa1 = oaps.tile([65, 512], F32, tag="oa", name=f"oa1_{hp}_{j}")
        for t in range(ST):
            sc = scps.tile([128, 1024], F32, tag="sc")
            et = ap_.tile([128, 1024], MM_DT, tag="et", bufs=2)
            for h in range(2):
                nc.tensor.matmul(
                    sc[:, h * 512:(h + 1) * 512],
                    kt[h * 64:(h + 1) * 64, t * 128:(t + 1) * 128],
                    qt[h * 64:(h + 1) * 64, j * 512:(j + 1) * 512],
                    start=True, stop=True,
                    tile_position=(h * 64, 0))
            nc.scalar.activation(et[:], sc[:], EXP)
            if pre_vmm:
                pre_vmm.pop(0)()
            if pending is not None:
                vmm(*pending)
            pending = (et, oa0, oa1, t, j)
    vmm(*pending)


def _shard_inputs(x, W_qkv, b_qkv, W_out, b_out):
    """Build the 8 per-core input maps. Attention scale folded into Wq/bq."""
    scale = np.float32(HD ** -0.5)
    in_maps = []
    for c in range(8):
        b, g = c // TP, c % TP
        lo, hi = g * VW, (g + 1) * VW
        wq = W_qkv[:, lo:hi] * scale
        wk = W_qkv[:, E + lo:E + hi]
        wv = W_qkv[:, 2 * E + lo:2 * E + hi]
        bq = b_qkv[lo:hi] * scale
        bk = b_qkv[E + lo:E + hi]
        bv = b_qkv[2 * E + lo:2 * E + hi]
        bqk = np.concatenate([bq, bk]).reshape(MT, 128).T
        in_maps.append({
            "xT": np.ascontiguousarray(x[b].T),
            "wqk": np.ascontiguousarray(np.concatenate([wq, wk], axis=1)),
            "bqk": np.ascontiguousarray(bqk),
            "wv": np.ascontiguousarray(wv),
            "bv": np.ascontiguousarray(bv[None, :]),
            "wo": np.ascontiguousarray(W_out[lo:hi, :]),
        })
    return in_maps



def kernel(x, W_qkv, b_qkv, W_out, b_out):
    x = np.asarray(x, dtype=np.float32)
    W_qkv = np.asarray(W_qkv, dtype=np.float32)
    b_qkv = np.asarray(b_qkv, dtype=np.float32)
    W_out = np.asarray(W_out, dtype=np.float32)
    b_out = np.asarray(b_out, dtype=np.float32)
    if "nc" not in _CACHE:
        _CACHE["nc"] = build_nc()
    nc = _CACHE["nc"]
    in_maps = _shard_inputs(x, W_qkv, b_qkv, W_out, b_out)
    res = None
    for attempt in range(3):
        try:
            res = run_bass_kernel_spmd(nc, in_maps, core_ids=list(range(8)))
            break
        except Exception:
            if attempt == 2:
                raise
    _CACHE["last_results"] = res
    out = np.empty((B, S, E), dtype=np.float32)
    for b in range(B):
        out[b] = res.results[TP * b]["out"] + res.results[TP * b + 1]["out"] + b_out
    return out



# revision 5
# speedup vs baseline: 1.0719x; 1.0719x over previous
"""Trainium2 Bass kernel for 16-head MHA (E=1024, S=2048, B=4) on 8 NeuronCores.

Sharding: tensor-parallel over head groups (TP=2: heads 0-7 / 8-15) x
data-parallel over batch (DP=4).  Core c handles batch c//2, head group c%2.
Host sums the two TP out-projection partials and adds b_out.

Device dataflow per core (per-stage dtypes chosen from an error budget):
  qk proj : bf16 matmuls -> PSUM; PSUM->SBUF copies emit a DITHER PAIR of
            e4m3 tensors q*4(1+h) / q*4(1-h), h=2^-5 (bias fused per-partition).
  v proj  : bf16 matmuls in transposed layout [vcol, t] (bias per-partition),
            fp16 result DMA-transposed into vaug [t, ti, head, 64+ones].
  scores  : fp8e4m3 DoubleRow matmuls, the 2 DR k-subtiles = the dither pair;
            first-order quantization error cancels.  out = scores^T [t, s].
  exp     : ACT exact exp->fp16 for most t-tiles; DVE Schraudolph
            (rint(z*c1+c2) as int16 bits == fp16 exp) for the rest.
  attn@V  : fp16 matmuls, et stationary [t, s-chunk], V moving [t, 64+1];
            out [s,65] accumulates over t in 4-chain PSUM banks; col 64 is
            the softmax denominator (ones column of vaug).
  norm    : per-partition reciprocal + tensor_scalar mult -> O bf16 [s, hd],
            head pairs packed [128 s, 128 hd], DMA-transposed into ot blocks.
  out proj: bf16 matmuls, ot stationary [hd, s-tile], W_out moving -> out
            [s, e] bf16, DMA'd per s-tile.
"""

import numpy as np
import ml_dtypes

import concourse.bass as bass
import concourse.tile as tile
from concourse import bacc, mybir
from concourse.alu_op_type import AluOpType
from concourse.bass_utils import run_bass_kernel_spmd

F32 = mybir.dt.float32
F16 = mybir.dt.float16
BF16 = mybir.dt.bfloat16
FP8 = mybir.dt.float8e4
I16 = mybir.dt.int16
EXP = mybir.ActivationFunctionType.Exp
IDENT = mybir.ActivationFunctionType.Identity
COPY = mybir.ActivationFunctionType.Copy
DR = mybir.MatmulPerfMode.DoubleRow

E = 1024          # embed dim
S = 2048          # sequence
B = 4             # batch
NH = 16           # total heads
HD = 64           # head dim
TP = 2            # head-group shards
HPC = NH // TP    # heads per core = 8
VW = HPC * HD     # 512 v columns per core
KCH = E // 128    # 8 contraction chunks

DITH = 2.0 ** -5                      # dither half-width
SP_, SM_ = 4.0 * (1 + DITH), 4.0 * (1 - DITH)
ESC = 1.0 / (256.0 * (1.0 + DITH * DITH))   # exp input scale (incl 1/sqrt(64))
SC1 = 1024.0 / np.log(2.0)            # fp16 schraudolph mult
SC2 = 15.0 * 1024.0 - 44.0            # fp16 schraudolph offset (tuned)
# exp engine per t-tile index: True -> DVE schraudolph, False -> ACT exact
EXPMAP = [False, False, False, True] * 4

_CACHE = {}


def build_nc():
    nc = bacc.Bacc("TRN2", target_bir_lowering=False, debug=False,
                   num_devices=8)

    xb_d = nc.dram_tensor("xb", [128, KCH, S], BF16, kind="ExternalInput").ap()
    wqk_d = nc.dram_tensor("wqk", [128, 8, KCH, 128], BF16,
                           kind="ExternalInput").ap()
    bqkp_d = nc.dram_tensor("bqkp", [128, 8], F32, kind="ExternalInput").ap()
    bqkm_d = nc.dram_tensor("bqkm", [128, 8], F32, kind="ExternalInput").ap()
    wv_d = nc.dram_tensor("wv", [128, KCH, VW], BF16,
                          kind="ExternalInput").ap()
    bv_d = nc.dram_tensor("bv", [128, 4], F32, kind="ExternalInput").ap()
    wo_d = nc.dram_tensor("wo", [128, 4, E], BF16, kind="ExternalInput").ap()
    out_d = nc.dram_tensor("out", [S, E], BF16, kind="ExternalOutput").ap()

    with tile.TileContext(nc) as tc:
        with (tc.tile_pool(name="persist", bufs=1) as pp,
              tc.tile_pool(name="bps", bufs=2, space="PSUM") as bps,
              tc.tile_pool(name="scps", bufs=2, space="PSUM") as scps,
              tc.tile_pool(name="oaps", bufs=2, space="PSUM") as oaps,
              tc.tile_pool(name="etp", bufs=10) as etp,
              tc.tile_pool(name="vst", bufs=2) as vst,
              tc.tile_pool(name="opr", bufs=2) as opr,
              tc.tile_pool(name="rcpp", bufs=4) as rcpp,
              tc.tile_pool(name="outp", bufs=3) as outp):
            xb = pp.tile([128, KCH, S], BF16, tag="xb")
            wqk = pp.tile([128, 8, KCH, 128], BF16, tag="wqk")
            bqkp = pp.tile([128, 8], F32, tag="bqkp")
            bqkm = pp.tile([128, 8], F32, tag="bqkm")
            wv = pp.tile([128, KCH, VW], BF16, tag="wv")
            bv = pp.tile([128, 4], F32, tag="bv")
            wo = pp.tile([128, 4, E], BF16, tag="wo")
            vaug = pp.tile([128, 16, HPC, 65], F16, tag="vaug")
            kt = [pp.tile([128, 2, S], FP8, tag=f"kt{i}", name=f"kt{i}")
                  for i in range(4)]
            qt = [pp.tile([128, 2, S], FP8, tag=f"qt{i}", name=f"qt{i}")
                  for i in range(4)]
            ot = [pp.tile([128, S], BF16, tag=f"ot{i}", name=f"ot{i}")
                  for i in range(4)]

            nc.vector.memset(vaug[:, :, :, 64:65], 1.0)

            # ---- input DMAs (order matters: m=4, m=0, wv/x early) ----
            morder = [4, 0, 5, 1, 6, 2, 7, 3]
            for k in range(KCH):
                eng = nc.sync if k % 2 == 0 else nc.scalar
                eng.dma_start(xb[:, k, :], xb_d[:, k, :])
            nc.scalar.dma_start(wv[:], wv_d[:])
            for m in morder:
                eng = nc.sync if m % 2 == 0 else nc.scalar
                eng.dma_start(wqk[:, m, :, :], wqk_d[:, m, :, :])
            nc.sync.dma_start(bqkp[:], bqkp_d[:])
            nc.sync.dma_start(bqkm[:], bqkm_d[:])
            nc.scalar.dma_start(bv[:], bv_d[:])
            nc.scalar.dma_start(wo[:], wo_d[:])

            # ---- qk projection: m-tile -> dither pair in kt/qt ----
            def emit_qk(m, jlist=None):
                dest = kt[m - 4] if m >= 4 else qt[m]
                for j4 in jlist if jlist is not None else range(4):
                    ps = bps.tile([128, 512], F32, tag="ps",
                                  name=f"ps{m}_{j4}")
                    for k in range(KCH):
                        nc.tensor.matmul(
                            ps[:], wqk[:, m, k, :],
                            xb[:, k, j4 * 512:(j4 + 1) * 512],
                            start=(k == 0), stop=(k == KCH - 1))
                    sl = dest[:, 0, j4 * 512:(j4 + 1) * 512]
                    nc.vector.tensor_scalar(sl, ps[:], SP_, bqkp[:, m:m + 1],
                                            op0=AluOpType.mult,
                                            op1=AluOpType.add)
                    sl = dest[:, 1, j4 * 512:(j4 + 1) * 512]
                    nc.scalar.activation(sl, ps[:], IDENT,
                                         bias=bqkm[:, m:m + 1], scale=SM_)

            # ---- v projection (transposed layout), one vt = 2 heads ----
            def emit_v(vt):
                vsb = vst.tile([128, 512], F16, tag="vsb", name=f"vsb{vt}")
                for tc4 in range(4):
                    vp = bps.tile([128, 512], F32, tag="ps",
                                  name=f"vp{vt}_{tc4}")
                    for k in range(KCH):
                        nc.tensor.matmul(
                            vp[:], wv[:, k, vt * 128:(vt + 1) * 128],
                            xb[:, k, tc4 * 512:(tc4 + 1) * 512],
                            start=(k == 0), stop=(k == KCH - 1))
                    nc.vector.tensor_scalar(
                        vsb[:], vp[:], bv[:, vt:vt + 1], None,
                        op0=AluOpType.add)
                    sc4 = vst.tile([128, 4, 128], F16, tag="sc4",
                                   name=f"sc4_{vt}_{tc4}")
                    for tt in range(4):
                        nc.sync.dma_start_transpose(
                            sc4[:, tt, :], vsb[:, tt * 128:(tt + 1) * 128])
                    nc.vector.tensor_copy(
                        vaug[:, tc4 * 4:(tc4 + 1) * 4,
                             2 * vt:2 * vt + 2, 0:64],
                        sc4[:].rearrange("p a (b c) -> p a b c", b=2))

            # preamble: everything heads 0/1 need, then interleave the rest
            emit_qk(4)
            emit_v(0)
            emit_qk(0)
            proj_rest = [lambda: emit_qk(5), lambda: emit_qk(1),
                         lambda: emit_v(1), lambda: emit_qk(6),
                         lambda: emit_qk(2), lambda: emit_v(2),
                         lambda: emit_qk(7), lambda: emit_qk(3),
                         lambda: emit_v(3)]

            # ---- attention unit (h, jj): jj = s-half (1024 wide) ----
            def attn_unit(h, jj):
                hp, p0 = h // 2, (h % 2) * 64
                oa = [oaps.tile([128, 4, 65], F32, tag="oa",
                                name=f"oa{h}_{jj}_{g}") for g in range(2)]
                ets = [None] * 16
                for wave in range(2):
                    for tl in range(8):
                        ti = wave * 8 + tl
                        sc = scps.tile([128, 1024], F32, tag="sc",
                                       name=f"sc{h}_{jj}_{ti}")
                        for sh in range(2):
                            nc.tensor.matmul(
                                sc[:, sh * 512:(sh + 1) * 512],
                                kt[hp][p0:p0 + 64, :,
                                       ti * 128:(ti + 1) * 128],
                                qt[hp][p0:p0 + 64, :,
                                       jj * 1024 + sh * 512:
                                       jj * 1024 + (sh + 1) * 512],
                                start=True, stop=True, perf_mode=DR)
                        et = etp.tile([128, 1024], F16, tag="et",
                                      name=f"et{h}_{jj}_{ti}")
                        if EXPMAP[ti]:
                            nc.vector.tensor_scalar(
                                et[:].bitcast(I16), sc[:], ESC * SC1, SC2,
                                op0=AluOpType.mult, op1=AluOpType.add)
                        else:
                            nc.scalar.activation(et[:], sc[:], EXP, scale=ESC)
                        ets[ti] = et
                    for st in range(8):
                        for tl in range(8):
                            ti = wave * 8 + tl
                            nc.tensor.matmul(
                                oa[st // 4][:, st % 4, :],
                                ets[ti][:, st * 128:(st + 1) * 128],
                                vaug[:, ti, h, :],
                                start=(ti == 0), stop=(ti == 15))
                return oa

            opair = {}

            def drain_unit(h, jj, oa):
                hp = h // 2
                if (hp, jj) not in opair:
                    opair[(hp, jj)] = opr.tile([128, 8, 128], BF16, tag="op",
                                               name=f"op{hp}_{jj}")
                osb = opair[(hp, jj)]
                for st in range(8):
                    rcp = rcpp.tile([128, 1], F32, tag="rcp")
                    nc.vector.reciprocal(rcp[:], oa[st // 4][:, st % 4,
                                                             64:65])
                    nc.vector.tensor_scalar(
                        osb[:, st, (h % 2) * 64:(h % 2) * 64 + 64],
                        oa[st // 4][:, st % 4, 0:64], rcp[:], None,
                        op0=AluOpType.mult)
                if h % 2 == 1:
                    for st in range(8):
                        nc.sync.dma_start_transpose(
                            ot[hp][:, (jj * 8 + st) * 128:
                                   (jj * 8 + st + 1) * 128],
                            osb[:, st, :])
                    del opair[(hp, jj)]

            # ---- out projection for one s-tile ----
            def emit_out(st):
                osb = outp.tile([128, E], BF16, tag="outsb", name=f"os{st}")
                for e2 in range(2):
                    op = bps.tile([128, 512], F32, tag="ps",
                                  name=f"op{st}_{e2}")
                    for kc in range(4):
                        nc.tensor.matmul(
                            op[:], ot[kc][:, st * 128:(st + 1) * 128],
                            wo[:, kc, e2 * 512:(e2 + 1) * 512],
                            start=(kc == 0), stop=(kc == 3))
                    nc.scalar.activation(osb[:, e2 * 512:(e2 + 1) * 512],
                                         op[:], COPY)
                nc.sync.dma_start(out_d[st * 128:(st + 1) * 128, :], osb[:])

            # ---- main schedule: h-outer with projections interleaved
            # between units; out-proj as soon as the last head-pair's
            # transposes for an s-range have been emitted.
            pr = list(proj_rest)
            for h in range(HPC):
                for jj in range(2):
                    oa = attn_unit(h, jj)
                    drain_unit(h, jj, oa)
                    if pr:
                        pr.pop(0)()
                    if h == HPC - 1:
                        for st in range(jj * 8, jj * 8 + 8):
                            emit_out(st)

    nc.compile()
    return nc


def _shard_inputs(x, W_qkv, b_qkv, W_out, b_out):
    BF = ml_dtypes.bfloat16
    xbs = []
    for b in range(B):
        xT = np.ascontiguousarray(x[b].T)                       # [E, S]
        xbs.append(np.ascontiguousarray(
            xT.reshape(KCH, 128, S).transpose(1, 0, 2)).astype(BF))
    gshards = []
    for g in range(TP):
        lo, hi = g * VW, (g + 1) * VW
        Wq = W_qkv[:, lo:hi]
        Wk = W_qkv[:, E + lo:E + hi]
        Wv_ = W_qkv[:, 2 * E + lo:2 * E + hi]
        bq = b_qkv[lo:hi]
        bk = b_qkv[E + lo:E + hi]
        bvv = b_qkv[2 * E + lo:2 * E + hi]
        Wqk = np.concatenate([Wq, Wk], axis=1)                  # [E, 1024]
        wqk = np.ascontiguousarray(
            Wqk.reshape(KCH, 128, 8, 128).transpose(1, 2, 0, 3)).astype(BF)
        bcat = np.concatenate([bq, bk]).reshape(8, 128).T       # [128, 8]
        wv = np.ascontiguousarray(
            Wv_.reshape(KCH, 128, VW).transpose(1, 0, 2)).astype(BF)
        bvt = bvv.reshape(4, 128).T                             # [128, 4]
        wo = np.ascontiguousarray(
            W_out[lo:hi, :].reshape(4, 128, E).transpose(1, 0, 2)).astype(BF)
        gshards.append({
            "wqk": wqk,
            "bqkp": np.ascontiguousarray(bcat * SP_, dtype=np.float32),
            "bqkm": np.ascontiguousarray(bcat * SM_, dtype=np.float32),
            "wv": wv,
            "bv": np.ascontiguousarray(bvt, dtype=np.float32),
            "wo": wo,
        })
    in_maps = []
    for c in range(8):
        b, g = c // TP, c % TP
        m = dict(gshards[g])
        m["xb"] = xbs[b]
        in_maps.append(m)
    return in_maps


def kernel(x, W_qkv, b_qkv, W_out, b_out):
    x = np.asarray(x, dtype=np.float32)
    W_qkv = np.asarray(W_qkv, dtype=np.float32)
    b_qkv = np.asarray(b_qkv, dtype=np.float32)
    W_out = np.asarray(W_out, dtype=np.float32)
    b_out = np.asarray(b_out, dtype=np.float32)
    if "nc" not in _CACHE:
        _CACHE["nc"] = build_nc()
    nc = _CACHE["nc"]
    in_maps = _shard_inputs(x, W_qkv, b_qkv, W_out, b_out)
    res = None
    for attempt in range(3):
        try:
            res = run_bass_kernel_spmd(nc, in_maps, core_ids=list(range(8)))
            break
        except Exception:
            if attempt == 2:
                raise
    _CACHE["last_results"] = res
    out = np.empty((B, S, E), dtype=np.float32)
    for b in range(B):
        out[b] = (res.results[TP * b]["out"].astype(np.float32) +
                  res.results[TP * b + 1]["out"].astype(np.float32) + b_out)
    return out


# revision 12
# speedup vs baseline: 1.2168x; 1.1352x over previous
"""Trainium2 Bass kernel for 16-head MHA (E=1024, S=2048, B=4) on 8 NeuronCores.

Sharding: tensor-parallel over head groups (TP=2: heads 0-7 / 8-15) x
data-parallel over batch (DP=4).  Core c handles batch c//2, head group c%2.
Host sums the two TP out-projection partials and adds b_out.

Device dataflow per core (per-stage dtypes chosen from an error budget):
  qk proj : bf16 matmuls -> PSUM; PSUM->SBUF copies emit a DITHER PAIR of
            e4m3 tensors q*4(1+h) / q*4(1-h), h=2^-5 (bias fused per-partition).
  v proj  : bf16 matmuls in transposed layout [vcol, t] (bias per-partition),
            fp16 result DMA-transposed into vaug [t, ti, head, 64+ones].
  scores  : fp8e4m3 DoubleRow matmuls, the 2 DR k-subtiles = the dither pair;
            first-order quantization error cancels.  out = scores^T [t, s].
  exp     : ACT exact exp->fp16 for most t-tiles; DVE Schraudolph
            (rint(z*c1+c2) as int16 bits == fp16 exp) for the rest.
  attn@V  : fp16 matmuls, et stationary [t, s-chunk], V moving [t, 64+1];
            out [s,65] accumulates over t in 4-chain PSUM banks; col 64 is
            the softmax denominator (ones column of vaug).
  norm    : per-partition reciprocal + tensor_scalar mult -> O bf16 [s, hd],
            head pairs packed [128 s, 128 hd], DMA-transposed into ot blocks.
  out proj: bf16 matmuls, ot stationary [hd, s-tile], W_out moving -> out
            [s, e] bf16, DMA'd per s-tile.
"""

import numpy as np
import ml_dtypes

import concourse.bass as bass
import concourse.tile as tile
from concourse import bacc, mybir
from concourse.alu_op_type import AluOpType
from concourse.bass_utils import run_bass_kernel_spmd

F32 = mybir.dt.float32
F16 = mybir.dt.float16
BF16 = mybir.dt.bfloat16
FP8 = mybir.dt.float8e4
I16 = mybir.dt.int16
EXP = mybir.ActivationFunctionType.Exp
IDENT = mybir.ActivationFunctionType.Identity
COPY = mybir.ActivationFunctionType.Copy
DR = mybir.MatmulPerfMode.DoubleRow

E = 1024          # embed dim
S = 2048          # sequence
B = 4             # batch
NH = 16           # total heads
HD = 64           # head dim
TP = 2            # head-group shards
HPC = NH // TP    # heads per core = 8
VW = HPC * HD     # 512 v columns per core
KCH = E // 128    # 8 contraction chunks

DITH = 2.0 ** -5                      # dither half-width
SP_, SM_ = 4.0 * (1 + DITH), 4.0 * (1 - DITH)
ESC = 1.0 / (256.0 * (1.0 + DITH * DITH))   # exp input scale (incl 1/sqrt(64))
SC1 = 1024.0 / np.log(2.0)            # fp16 schraudolph mult
SC2 = 15.0 * 1024.0 - 44.0            # fp16 schraudolph offset (tuned)
# exp engine per t-tile index: True -> DVE schraudolph, False -> ACT exact
EXPMAP = [False, False, True, False, False, True, False, True] * 2

_CACHE = {}


def build_nc():
    nc = bacc.Bacc("TRN2", target_bir_lowering=False, debug=False,
                   num_devices=8)

    xb_d = nc.dram_tensor("xb", [128, KCH, S], BF16, kind="ExternalInput").ap()
    wqk_d = nc.dram_tensor("wqk", [128, 8, KCH, 128], BF16,
                           kind="ExternalInput").ap()
    bqkp_d = nc.dram_tensor("bqkp", [128, 8], F32, kind="ExternalInput").ap()
    bqkm_d = nc.dram_tensor("bqkm", [128, 8], F32, kind="ExternalInput").ap()
    wv_d = nc.dram_tensor("wv", [128, KCH, VW], BF16,
                          kind="ExternalInput").ap()
    bv_d = nc.dram_tensor("bv", [128, 4], F32, kind="ExternalInput").ap()
    wo_d = nc.dram_tensor("wo", [128, 4, E], BF16, kind="ExternalInput").ap()
    out_d = nc.dram_tensor("out", [S, E], BF16, kind="ExternalOutput").ap()

    with tile.TileContext(nc) as tc:
        with (tc.tile_pool(name="persist", bufs=1) as pp,
              tc.tile_pool(name="bps", bufs=2, space="PSUM") as bps,
              tc.tile_pool(name="scps", bufs=2, space="PSUM") as scps,
              tc.tile_pool(name="oaps", bufs=2, space="PSUM") as oaps,
              tc.tile_pool(name="etp", bufs=10) as etp,
              tc.tile_pool(name="vst", bufs=2) as vst,
              tc.tile_pool(name="opr", bufs=2) as opr,
              tc.tile_pool(name="rcpp", bufs=4) as rcpp,
              tc.tile_pool(name="outp", bufs=3) as outp):
            xb = pp.tile([128, KCH, S], BF16, tag="xb")
            wqk = pp.tile([128, 8, KCH, 128], BF16, tag="wqk")
            bqkp = pp.tile([128, 8], F32, tag="bqkp")
            bqkm = pp.tile([128, 8], F32, tag="bqkm")
            wv = pp.tile([128, KCH, VW], BF16, tag="wv")
            bv = pp.tile([128, 4], F32, tag="bv")
            wo = pp.tile([128, 4, E], BF16, tag="wo")
            vaug = pp.tile([128, 16, HPC, 65], F16, tag="vaug")
            kt = [pp.tile([128, 2, S], FP8, tag=f"kt{i}", name=f"kt{i}")
                  for i in range(4)]
            qt = [pp.tile([128, 2, S], FP8, tag=f"qt{i}", name=f"qt{i}")
                  for i in range(4)]
            ot = [pp.tile([128, S], BF16, tag=f"ot{i}", name=f"ot{i}")
                  for i in range(4)]

            nc.vector.memset(vaug[:, :, :, 64:65], 1.0)

            # ---- input DMAs (order matters: m=4, m=0, wv/x early) ----
            morder = [4, 0, 5, 1, 6, 2, 7, 3]
            for k in range(KCH):
                eng = nc.sync if k % 2 == 0 else nc.scalar
                eng.dma_start(xb[:, k, :], xb_d[:, k, :])
            nc.scalar.dma_start(wv[:], wv_d[:])
            for m in morder:
                eng = nc.sync if m % 2 == 0 else nc.scalar
                eng.dma_start(wqk[:, m, :, :], wqk_d[:, m, :, :])
            nc.sync.dma_start(bqkp[:], bqkp_d[:])
            nc.sync.dma_start(bqkm[:], bqkm_d[:])
            nc.scalar.dma_start(bv[:], bv_d[:])
            nc.scalar.dma_start(wo[:], wo_d[:])

            # ---- qk projection: m-tile -> dither pair in kt/qt ----
            def emit_qk(m, jlist=None):
                dest = kt[m - 4] if m >= 4 else qt[m]
                for j4 in jlist if jlist is not None else range(4):
                    ps = bps.tile([128, 512], F32, tag="ps",
                                  name=f"ps{m}_{j4}")
                    for k in range(KCH):
                        nc.tensor.matmul(
                            ps[:], wqk[:, m, k, :],
                            xb[:, k, j4 * 512:(j4 + 1) * 512],
                            start=(k == 0), stop=(k == KCH - 1))
                    sl = dest[:, 0, j4 * 512:(j4 + 1) * 512]
                    nc.vector.tensor_scalar(sl, ps[:], SP_, bqkp[:, m:m + 1],
                                            op0=AluOpType.mult,
                                            op1=AluOpType.add)
                    sl = dest[:, 1, j4 * 512:(j4 + 1) * 512]
                    nc.scalar.activation(sl, ps[:], IDENT,
                                         bias=bqkm[:, m:m + 1], scale=SM_)

            # ---- v projection (transposed layout), one vt = 2 heads ----
            def emit_v_quarter(vt, tc4):
                vsb = vst.tile([128, 512], F16, tag="vsb",
                               name=f"vsb{vt}_{tc4}")
                vp = bps.tile([128, 512], F32, tag="ps",
                              name=f"vp{vt}_{tc4}")
                for k in range(KCH):
                    nc.tensor.matmul(
                        vp[:], wv[:, k, vt * 128:(vt + 1) * 128],
                        xb[:, k, tc4 * 512:(tc4 + 1) * 512],
                        start=(k == 0), stop=(k == KCH - 1))
                nc.vector.tensor_scalar(
                    vsb[:], vp[:], bv[:, vt:vt + 1], None,
                    op0=AluOpType.add)
                sc4 = vst.tile([128, 4, 128], F16, tag="sc4",
                               name=f"sc4_{vt}_{tc4}")
                nc.sync.dma_start_transpose(sc4[:], vsb[:])
                nc.vector.tensor_copy(
                    vaug[:, tc4 * 4:(tc4 + 1) * 4, 2 * vt:2 * vt + 2, 0:64],
                    sc4[:].rearrange("p a (b c) -> p a b c", b=2))

            def emit_v(vt):
                for tc4 in range(4):
                    emit_v_quarter(vt, tc4)

            # preamble: everything heads 0/1 need, then interleave the rest
            # as fine-grained filler pieces popped inside attention units
            emit_qk(4)
            emit_v(0)
            emit_qk(0)
            filler = []
            for m, vt in [(5, None), (1, None), (None, 1), (6, None),
                          (2, None), (None, 2), (7, None), (3, None),
                          (None, 3)]:
                if m is not None:
                    filler += [lambda m=m, j=j: emit_qk(m, [j])
                               for j in range(4)]
                else:
                    filler += [lambda v=vt, t=t: emit_v_quarter(v, t)
                               for t in range(4)]

            def pop_filler():
                if filler:
                    filler.pop(0)()

            # ---- attention unit (h, jj): jj = s-half (1024 wide).
            # wave-0 V-matmuls interleave with wave-1 scores; wave-1
            # V-matmuls trail (they overlap the next unit's scores).
            def attn_unit(h, jj):
                hp, p0 = h // 2, (h % 2) * 64
                oa = [oaps.tile([128, 4, 65], F32, tag="oa",
                                name=f"oa{h}_{jj}_{g}") for g in range(2)]
                ets = [None] * 16

                def scores_one(ti):
                    sc = scps.tile([128, 1024], F32, tag="sc",
                                   name=f"sc{h}_{jj}_{ti}")
                    for sh in range(2):
                        nc.tensor.matmul(
                            sc[:, sh * 512:(sh + 1) * 512],
                            kt[hp][p0:p0 + 64, :, ti * 128:(ti + 1) * 128],
                            qt[hp][p0:p0 + 64, :,
                                   jj * 1024 + sh * 512:
                                   jj * 1024 + (sh + 1) * 512],
                            start=True, stop=True, perf_mode=DR)
                    et = etp.tile([128, 1024], F16, tag="et",
                                  name=f"et{h}_{jj}_{ti}")
                    if EXPMAP[ti]:
                        nc.vector.tensor_scalar(
                            et[:].bitcast(I16), sc[:], ESC * SC1, SC2,
                            op0=AluOpType.mult, op1=AluOpType.add)
                    else:
                        nc.scalar.activation(et[:], sc[:], EXP, scale=ESC)
                    ets[ti] = et

                def vmm(st, ti):
                    nc.tensor.matmul(
                        oa[st // 4][:, st % 4, :],
                        ets[ti][:, st * 128:(st + 1) * 128],
                        vaug[:, ti, h, :],
                        start=(ti == 0), stop=(ti == 15))

                for tl in range(8):
                    scores_one(tl)
                    if tl % 4 == 3:
                        pop_filler()
                for tl in range(8):
                    scores_one(8 + tl)
                    for t0 in range(8):
                        vmm(tl, t0)
                    if tl % 4 == 3:
                        pop_filler()
                for st in range(8):
                    for tl in range(8):
                        vmm(st, 8 + tl)
                return oa

            opair = {}

            def drain_unit(h, jj, oa):
                hp = h // 2
                if (hp, jj) not in opair:
                    opair[(hp, jj)] = opr.tile([128, 8, 128], BF16, tag="op",
                                               name=f"op{hp}_{jj}")
                osb = opair[(hp, jj)]
                for st in range(8):
                    rcp = rcpp.tile([128, 1], F32, tag="rcp")
                    nc.vector.reciprocal(rcp[:], oa[st // 4][:, st % 4,
                                                             64:65])
                    nc.vector.tensor_scalar(
                        osb[:, st, (h % 2) * 64:(h % 2) * 64 + 64],
                        oa[st // 4][:, st % 4, 0:64], rcp[:], None,
                        op0=AluOpType.mult)
                if h % 2 == 1:
                    nc.sync.dma_start_transpose(
                        ot[hp][:, jj * 1024:(jj + 1) * 1024].rearrange(
                            "p (a c) -> p a c", c=128),
                        osb[:].rearrange("p a c -> p (a c)"))
                    del opair[(hp, jj)]

            # ---- out projection for one s-tile ----
            def emit_out(st):
                osb = outp.tile([128, E], BF16, tag="outsb", name=f"os{st}")
                for e2 in range(2):
                    op = bps.tile([128, 512], F32, tag="ps",
                                  name=f"op{st}_{e2}")
                    for kc in range(4):
                        nc.tensor.matmul(
                            op[:], ot[kc][:, st * 128:(st + 1) * 128],
                            wo[:, kc, e2 * 512:(e2 + 1) * 512],
                            start=(kc == 0), stop=(kc == 3))
                    nc.scalar.activation(osb[:, e2 * 512:(e2 + 1) * 512],
                                         op[:], COPY)
                nc.gpsimd.dma_start(out_d[st * 128:(st + 1) * 128, :],
                                    osb[:])

            # ---- main schedule: h-outer, projections drip-fed as filler;
            # out-proj as soon as the last head-pair's transposes for an
            # s-range have been emitted.
            for h in range(HPC):
                for jj in range(2):
                    oa = attn_unit(h, jj)
                    drain_unit(h, jj, oa)
                    if h == HPC - 1:
                        for st in range(jj * 8, jj * 8 + 8):
                            emit_out(st)
            while filler:
                filler.pop(0)()

    nc.compile()
    return nc


def _shard_inputs(x, W_qkv, b_qkv, W_out, b_out):
    BF = ml_dtypes.bfloat16
    xbs = []
    for b in range(B):
        xT = np.ascontiguousarray(x[b].T)                       # [E, S]
        xbs.append(np.ascontiguousarray(
            xT.reshape(KCH, 128, S).transpose(1, 0, 2)).astype(BF))
    gshards = []
    for g in range(TP):
        lo, hi = g * VW, (g + 1) * VW
        Wq = W_qkv[:, lo:hi]
        Wk = W_qkv[:, E + lo:E + hi]
        Wv_ = W_qkv[:, 2 * E + lo:2 * E + hi]
        bq = b_qkv[lo:hi]
        bk = b_qkv[E + lo:E + hi]
        bvv = b_qkv[2 * E + lo:2 * E + hi]
        Wqk = np.concatenate([Wq, Wk], axis=1)                  # [E, 1024]
        wqk = np.ascontiguousarray(
            Wqk.reshape(KCH, 128, 8, 128).transpose(1, 2, 0, 3)).astype(BF)
        bcat = np.concatenate([bq, bk]).reshape(8, 128).T       # [128, 8]
        wv = np.ascontiguousarray(
            Wv_.reshape(KCH, 128, VW).transpose(1, 0, 2)).astype(BF)
        bvt = bvv.reshape(4, 128).T                             # [128, 4]
        wo = np.ascontiguousarray(
            W_out[lo:hi, :].reshape(4, 128, E).transpose(1, 0, 2)).astype(BF)
        gshards.append({
            "wqk": wqk,
            "bqkp": np.ascontiguousarray(bcat * SP_, dtype=np.float32),
            "bqkm": np.ascontiguousarray(bcat * SM_, dtype=np.float32),
            "wv": wv,
            "bv": np.ascontiguousarray(bvt, dtype=np.float32),
            "wo": wo,
        })
    in_maps = []
    for c in range(8):
        b, g = c // TP, c % TP
        m = dict(gshards[g])
        m["xb"] = xbs[b]
        in_maps.append(m)
    return in_maps


def kernel(x, W_qkv, b_qkv, W_out, b_out):
    x = np.asarray(x, dtype=np.float32)
    W_qkv = np.asarray(W_qkv, dtype=np.float32)
    b_qkv = np.asarray(b_qkv, dtype=np.float32)
    W_out = np.asarray(W_out, dtype=np.float32)
    b_out = np.asarray(b_out, dtype=np.float32)
    if "nc" not in _CACHE:
        _CACHE["nc"] = build_nc()
    nc = _CACHE["nc"]
    in_maps = _shard_inputs(x, W_qkv, b_qkv, W_out, b_out)
    res = None
    for attempt in range(3):
        try:
            res = run_bass_kernel_spmd(nc, in_maps, core_ids=list(range(8)))
            break
        except Exception:
            if attempt == 2:
                raise
    _CACHE["last_results"] = res
    out = np.empty((B, S, E), dtype=np.float32)
    for b in range(B):
        out[b] = (res.results[TP * b]["out"].astype(np.float32) +
                  res.results[TP * b + 1]["out"].astype(np.float32) + b_out)
    return out


# revision 20
# speedup vs baseline: 1.2554x; 1.0317x over previous
"""Trainium2 Bass kernel for 16-head MHA (E=1024, S=2048, B=4) on 8 NeuronCores.

Sharding: tensor-parallel over head groups (TP=2: heads 0-7 / 8-15) x
data-parallel over batch (DP=4).  Core c handles batch c//2, head group c%2.
Host sums the two TP out-projection partials and adds b_out.

Device dataflow per core (per-stage dtypes chosen from an error budget):
  qk proj : bf16 matmuls -> PSUM; PSUM->SBUF copies emit a DITHER PAIR of
            e4m3 tensors q*4(1+h) / q*4(1-h), h=2^-5 (bias fused per-partition).
  v proj  : bf16 matmuls in transposed layout [vcol, t] (bias per-partition),
            fp16 result DMA-transposed into vaug [t, ti, head, 64+ones].
  scores  : fp8e4m3 DoubleRow matmuls, the 2 DR k-subtiles = the dither pair;
            first-order quantization error cancels.  out = scores^T [t, s].
  exp     : ACT exact exp->fp16 for most t-tiles; DVE Schraudolph
            (rint(z*c1+c2) as int16 bits == fp16 exp) for the rest.
  attn@V  : fp16 matmuls, et stationary [t, s-chunk], V moving [t, 64+1];
            out [s,65] accumulates over t in 4-chain PSUM banks; col 64 is
            the softmax denominator (ones column of vaug).
  norm    : per-partition reciprocal + tensor_scalar mult -> O bf16 [s, hd],
            head pairs packed [128 s, 128 hd], DMA-transposed into ot blocks.
  out proj: bf16 matmuls, ot stationary [hd, s-tile], W_out moving -> out
            [s, e] bf16, DMA'd per s-tile.
"""

import numpy as np
import ml_dtypes

import concourse.bass as bass
import concourse.tile as tile
from concourse import bacc, mybir
from concourse.alu_op_type import AluOpType
from concourse.bass_utils import run_bass_kernel_spmd

F32 = mybir.dt.float32
F16 = mybir.dt.float16
BF16 = mybir.dt.bfloat16
FP8 = mybir.dt.float8e4
I16 = mybir.dt.int16
EXP = mybir.ActivationFunctionType.Exp
IDENT = mybir.ActivationFunctionType.Identity
COPY = mybir.ActivationFunctionType.Copy
DR = mybir.MatmulPerfMode.DoubleRow

E = 1024          # embed dim
S = 2048          # sequence
B = 4             # batch
NH = 16           # total heads
HD = 64           # head dim
TP = 2            # head-group shards
HPC = NH // TP    # heads per core = 8
VW = HPC * HD     # 512 v columns per core
KCH = E // 128    # 8 contraction chunks

DITH = 2.0 ** -5                      # dither half-width
SP_, SM_ = 4.0 * (1 + DITH), 4.0 * (1 - DITH)
ESC = 1.0 / (256.0 * (1.0 + DITH * DITH))   # exp input scale (incl 1/sqrt(64))
SC1 = 1024.0 / np.log(2.0)            # fp16 schraudolph mult
SC2 = 15.0 * 1024.0 - 44.0            # fp16 schraudolph offset (tuned)
# exp engine per t-tile index: True -> DVE schraudolph, False -> ACT exact.
# Early units have PE-filler so ACT leads; late units split closer to even.
EXPMAP_EARLY = [False, False, True, False, False, True, False, True] * 2
EXPMAP_LATE = [False, True, False, True, False, True, False, True] * 2

_CACHE = {}


def build_nc():
    nc = bacc.Bacc("TRN2", target_bir_lowering=False, debug=False,
                   num_devices=8)

    xb_d = nc.dram_tensor("xb", [128, KCH, S], BF16, kind="ExternalInput").ap()
    wqk_d = nc.dram_tensor("wqk", [128, 8, KCH, 128], BF16,
                           kind="ExternalInput").ap()
    bqkp_d = nc.dram_tensor("bqkp", [128, 8], F32, kind="ExternalInput").ap()
    bqkm_d = nc.dram_tensor("bqkm", [128, 8], F32, kind="ExternalInput").ap()
    wv_d = nc.dram_tensor("wv", [128, KCH, VW], BF16,
                          kind="ExternalInput").ap()
    bv_d = nc.dram_tensor("bv", [128, 4], F32, kind="ExternalInput").ap()
    wo_d = nc.dram_tensor("wo", [128, 4, E], BF16, kind="ExternalInput").ap()
    out_d = nc.dram_tensor("out", [S, E], BF16, kind="ExternalOutput").ap()

    with tile.TileContext(nc) as tc:
        with (tc.tile_pool(name="persist", bufs=1) as pp,
              tc.tile_pool(name="bps", bufs=2, space="PSUM") as bps,
              tc.tile_pool(name="scps", bufs=2, space="PSUM") as scps,
              tc.tile_pool(name="oaps", bufs=2, space="PSUM") as oaps,
              tc.tile_pool(name="etp", bufs=10) as etp,
              tc.tile_pool(name="vst", bufs=2) as vst,
              tc.tile_pool(name="opr", bufs=2) as opr,
              tc.tile_pool(name="rcpp", bufs=4) as rcpp,
              tc.tile_pool(name="outp", bufs=3) as outp):
            xb = pp.tile([128, KCH, S], BF16, tag="xb")
            wqk = pp.tile([128, 8, KCH, 128], BF16, tag="wqk")
            bqkp = pp.tile([128, 8], F32, tag="bqkp")
            bqkm = pp.tile([128, 8], F32, tag="bqkm")
            wv = pp.tile([128, KCH, VW], BF16, tag="wv")
            bv = pp.tile([128, 4], F32, tag="bv")
            wo = pp.tile([128, 4, E], BF16, tag="wo")
            vaug = pp.tile([128, 16, HPC, 65], F16, tag="vaug")
            kt = [pp.tile([128, 2, S], FP8, tag=f"kt{i}", name=f"kt{i}")
                  for i in range(4)]
            qt = [pp.tile([128, 2, S], FP8, tag=f"qt{i}", name=f"qt{i}")
                  for i in range(4)]
            ot = [pp.tile([128, S], BF16, tag=f"ot{i}", name=f"ot{i}")
                  for i in range(4)]

            nc.vector.memset(vaug[:, :, :, 64:65], 1.0)

            # ---- input DMAs (order matters: m=4, m=0, wv/x early) ----
            morder = [4, 0, 5, 1, 6, 2, 7, 3]
            for k in range(KCH):
                eng = nc.sync if k % 2 == 0 else nc.scalar
                eng.dma_start(xb[:, k, :], xb_d[:, k, :])
            nc.sync.dma_start(wqk[:, 4, :, :], wqk_d[:, 4, :, :])
            nc.scalar.dma_start(wqk[:, 0, :, :], wqk_d[:, 0, :, :])
            nc.sync.dma_start(bqkp[:], bqkp_d[:])
            nc.sync.dma_start(bqkm[:], bqkm_d[:])
            nc.scalar.dma_start(bv[:], bv_d[:])
            nc.scalar.dma_start(wv[:], wv_d[:])
            for m in morder[2:]:
                eng = nc.sync if m % 2 == 0 else nc.scalar
                eng.dma_start(wqk[:, m, :, :], wqk_d[:, m, :, :])
            nc.scalar.dma_start(wo[:], wo_d[:])

            # ---- qk projection: m-tile -> dither pair in kt/qt ----
            def emit_qk(m, jlist=None):
                dest = kt[m - 4] if m >= 4 else qt[m]
                for j4 in jlist if jlist is not None else range(4):
                    ps = bps.tile([128, 512], F32, tag="ps",
                                  name=f"ps{m}_{j4}")
                    for k in range(KCH):
                        nc.tensor.matmul(
                            ps[:], wqk[:, m, k, :],
                            xb[:, k, j4 * 512:(j4 + 1) * 512],
                            start=(k == 0), stop=(k == KCH - 1))
                    sl = dest[:, 0, j4 * 512:(j4 + 1) * 512]
                    nc.vector.tensor_scalar(sl, ps[:], SP_, bqkp[:, m:m + 1],
                                            op0=AluOpType.mult,
                                            op1=AluOpType.add)
                    sl = dest[:, 1, j4 * 512:(j4 + 1) * 512]
                    nc.scalar.activation(sl, ps[:], IDENT,
                                         bias=bqkm[:, m:m + 1], scale=SM_)

            # ---- v projection (transposed layout), one vt = 2 heads ----
            def emit_v_quarter(vt, tc4):
                vsb = vst.tile([128, 512], F16, tag="vsb",
                               name=f"vsb{vt}_{tc4}")
                vp = bps.tile([128, 512], F32, tag="ps",
                              name=f"vp{vt}_{tc4}")
                for k in range(KCH):
                    nc.tensor.matmul(
                        vp[:], wv[:, k, vt * 128:(vt + 1) * 128],
                        xb[:, k, tc4 * 512:(tc4 + 1) * 512],
                        start=(k == 0), stop=(k == KCH - 1))
                nc.vector.tensor_scalar(
                    vsb[:], vp[:], bv[:, vt:vt + 1], None,
                    op0=AluOpType.add)
                sc4 = vst.tile([128, 4, 128], F16, tag="sc4",
                               name=f"sc4_{vt}_{tc4}")
                nc.sync.dma_start_transpose(sc4[:], vsb[:])
                nc.vector.tensor_copy(
                    vaug[:, tc4 * 4:(tc4 + 1) * 4, 2 * vt:2 * vt + 2, 0:64],
                    sc4[:].rearrange("p a (b c) -> p a b c", b=2))

            def emit_v(vt):
                for tc4 in range(4):
                    emit_v_quarter(vt, tc4)

            # preamble: minimal prefix for unit (h0, jj0) wave-0; the rest
            # drip-feeds as fine-grained filler popped inside attention units
            emit_qk(4)
            emit_qk(0, [0, 1])
            emit_v_quarter(0, 0)
            emit_v_quarter(0, 1)
            filler = [lambda: emit_v_quarter(0, 2), lambda: emit_v_quarter(0, 3),
                      lambda: emit_qk(0, [2]), lambda: emit_qk(0, [3])]
            for m, vt in [(5, None), (1, None), (None, 1), (6, None),
                          (2, None), (None, 2), (7, None), (3, None),
                          (None, 3)]:
                if m is not None:
                    filler += [lambda m=m, j=j: emit_qk(m, [j])
                               for j in range(4)]
                else:
                    filler += [lambda v=vt, t=t: emit_v_quarter(v, t)
                               for t in range(4)]

            def pop_filler():
                if filler:
                    filler.pop(0)()

            # ---- attention unit (h, jj): jj = s-half (1024 wide).
            # wave-0 V-matmuls interleave with wave-1 scores; wave-1
            # V-matmuls trail (they overlap the next unit's scores).
            def attn_unit(h, jj):
                hp, p0 = h // 2, (h % 2) * 64
                uidx = h * 2 + jj
                expmap = EXPMAP_EARLY if uidx < 9 else EXPMAP_LATE
                oa = [oaps.tile([128, 4, 65], F32, tag="oa",
                                name=f"oa{h}_{jj}_{g}") for g in range(2)]
                ets = [None] * 16

                def scores_one(ti):
                    sc = scps.tile([128, 1024], F32, tag="sc",
                                   name=f"sc{h}_{jj}_{ti}")
                    for sh in range(2):
                        nc.tensor.matmul(
                            sc[:, sh * 512:(sh + 1) * 512],
                            kt[hp][p0:p0 + 64, :, ti * 128:(ti + 1) * 128],
                            qt[hp][p0:p0 + 64, :,
                                   jj * 1024 + sh * 512:
                                   jj * 1024 + (sh + 1) * 512],
                            start=True, stop=True, perf_mode=DR)
                    et = etp.tile([128, 1024], F16, tag="et",
                                  name=f"et{h}_{jj}_{ti}")
                    if expmap[ti]:
                        nc.vector.tensor_scalar(
                            et[:].bitcast(I16), sc[:], ESC * SC1, SC2,
                            op0=AluOpType.mult, op1=AluOpType.add)
                    else:
                        nc.scalar.activation(et[:], sc[:], EXP, scale=ESC)
                    ets[ti] = et

                def vmm(st, ti):
                    nc.tensor.matmul(
                        oa[st // 4][:, st % 4, :],
                        ets[ti][:, st * 128:(st + 1) * 128],
                        vaug[:, ti, h, :],
                        start=(ti == 0), stop=(ti == 15))

                for tl in range(8):
                    scores_one(tl)
                    if tl % 4 == 3:
                        pop_filler()
                for tl in range(8):
                    scores_one(8 + tl)
                    for t0 in range(8):
                        vmm(tl, t0)
                    if tl == 3 or (tl == 7 and uidx < 4):
                        pop_filler()
                for st in range(8):
                    for tl in range(8):
                        vmm(st, 8 + tl)
                return oa

            opair = {}

            def drain_unit(h, jj, oa):
                hp = h // 2
                if (hp, jj) not in opair:
                    opair[(hp, jj)] = opr.tile([128, 8, 128], BF16, tag="op",
                                               name=f"op{hp}_{jj}")
                osb = opair[(hp, jj)]
                for st in range(8):
                    rcp = rcpp.tile([128, 1], F32, tag="rcp")
                    nc.vector.reciprocal(rcp[:], oa[st // 4][:, st % 4,
                                                             64:65])
                    dst = osb[:, st, (h % 2) * 64:(h % 2) * 64 + 64]
                    src = oa[st // 4][:, st % 4, 0:64]
                    if st % 2 == 0:
                        nc.vector.tensor_scalar(dst, src, rcp[:], None,
                                                op0=AluOpType.mult)
                    else:
                        nc.scalar.activation(dst, src, COPY, scale=rcp[:])
                if h % 2 == 1:
                    nc.sync.dma_start_transpose(
                        ot[hp][:, jj * 1024:(jj + 1) * 1024].rearrange(
                            "p (a c) -> p a c", c=128),
                        osb[:].rearrange("p a c -> p (a c)"))
                    del opair[(hp, jj)]

            # ---- out projection for one s-tile ----
            def emit_out(st):
                osb = outp.tile([128, E], BF16, tag="outsb", name=f"os{st}")
                for e2 in range(2):
                    op = bps.tile([128, 512], F32, tag="ps",
                                  name=f"op{st}_{e2}")
                    for kc in range(4):
                        nc.tensor.matmul(
                            op[:], ot[kc][:, st * 128:(st + 1) * 128],
                            wo[:, kc, e2 * 512:(e2 + 1) * 512],
                            start=(kc == 0), stop=(kc == 3))
                    if e2 == 0:
                        nc.scalar.activation(
                            osb[:, e2 * 512:(e2 + 1) * 512], op[:], COPY)
                    else:
                        nc.vector.tensor_copy(
                            osb[:, e2 * 512:(e2 + 1) * 512], op[:])
                nc.gpsimd.dma_start(out_d[st * 128:(st + 1) * 128, :],
                                    osb[:])

            # ---- main schedule: h-outer, projections drip-fed as filler;
            # out-proj as soon as the last head-pair's transposes for an
            # s-range have been emitted.
            for h in range(HPC):
                for jj in range(2):
                    oa = attn_unit(h, jj)
                    drain_unit(h, jj, oa)
                    if h == HPC - 1:
                        for st in range(jj * 8, jj * 8 + 8):
                            emit_out(st)
            while filler:
                filler.pop(0)()

    nc.compile()
    return nc


def _shard_inputs(x, W_qkv, b_qkv, W_out, b_out):
    BF = ml_dtypes.bfloat16
    xbs = []
    for b in range(B):
        xT = np.ascontiguousarray(x[b].T)                       # [E, S]
        xbs.append(np.ascontiguousarray(
            xT.reshape(KCH, 128, S).transpose(1, 0, 2)).astype(BF))
    gshards = []
    for g in range(TP):
        lo, hi = g * VW, (g + 1) * VW
        Wq = W_qkv[:, lo:hi]
        Wk = W_qkv[:, E + lo:E + hi]
        Wv_ = W_qkv[:, 2 * E + lo:2 * E + hi]
        bq = b_qkv[lo:hi]
        bk = b_qkv[E + lo:E + hi]
        bvv = b_qkv[2 * E + lo:2 * E + hi]
        Wqk = np.concatenate([Wq, Wk], axis=1)                  # [E, 1024]
        wqk = np.ascontiguousarray(
            Wqk.reshape(KCH, 128, 8, 128).transpose(1, 2, 0, 3)).astype(BF)
        bcat = np.concatenate([bq, bk]).reshape(8, 128).T       # [128, 8]
        wv = np.ascontiguousarray(
            Wv_.reshape(KCH, 128, VW).transpose(1, 0, 2)).astype(BF)
        bvt = bvv.reshape(4, 128).T                             # [128, 4]
        wo = np.ascontiguousarray(
            W_out[lo:hi, :].reshape(4, 128, E).transpose(1, 0, 2)).astype(BF)
        gshards.append({
            "wqk": wqk,
            "bqkp": np.ascontiguousarray(bcat * SP_, dtype=np.float32),
            "bqkm": np.ascontiguousarray(bcat * SM_, dtype=np.float32),
            "wv": wv,
            "bv": np.ascontiguousarray(bvt, dtype=np.float32),
            "wo": wo,
        })
    in_maps = []
    for c in range(8):
        b, g = c // TP, c % TP
        m = dict(gshards[g])
        m["xb"] = xbs[b]
        in_maps.append(m)
    return in_maps


def kernel(x, W_qkv, b_qkv, W_out, b_out):
    x = np.asarray(x, dtype=np.float32)
    W_qkv = np.asarray(W_qkv, dtype=np.float32)
    b_qkv = np.asarray(b_qkv, dtype=np.float32)
    W_out = np.asarray(W_out, dtype=np.float32)
    b_out = np.asarray(b_out, dtype=np.float32)
    if "nc" not in _CACHE:
        _CACHE["nc"] = build_nc()
    nc = _CACHE["nc"]
    in_maps = _shard_inputs(x, W_qkv, b_qkv, W_out, b_out)
    res = None
    for attempt in range(3):
        try:
            res = run_bass_kernel_spmd(nc, in_maps, core_ids=list(range(8)))
            break
        except Exception:
            if attempt == 2:
                raise
    _CACHE["last_results"] = res
    out = np.empty((B, S, E), dtype=np.float32)
    for b in range(B):
        out[b] = (res.results[TP * b]["out"].astype(np.float32) +
                  res.results[TP * b + 1]["out"].astype(np.float32) + b_out)
    return out


# revision 24
# speedup vs baseline: 1.3043x; 1.0389x over previous
"""Trainium2 Bass kernel for 16-head MHA (E=1024, S=2048, B=4) on 8 NeuronCores.

Sharding: tensor-parallel over head groups (TP=2: heads 0-7 / 8-15) x
data-parallel over batch (DP=4).  Core c handles batch c//2, head group c%2.
Host sums the two TP out-projection partials and adds b_out.

Device dataflow per core (per-stage dtypes chosen from an error budget):
  qk proj : bf16 matmuls -> PSUM; PSUM->SBUF copies emit a DITHER PAIR of
            e4m3 tensors q*4(1+h) / q*4(1-h), h=2^-5 (bias fused per-partition).
  v proj  : bf16 matmuls in transposed layout [vcol, t] (bias per-partition),
            fp16 result DMA-transposed into vaug [t, ti, head, 64+ones].
  scores  : fp8e4m3 DoubleRow matmuls, the 2 DR k-subtiles = the dither pair;
            first-order quantization error cancels.  out = scores^T [t, s].
  exp     : ACT exact exp->fp16 for most t-tiles; DVE Schraudolph
            (rint(z*c1+c2) as int16 bits == fp16 exp) for the rest.
  attn@V  : fp16 matmuls, et stationary [t, s-chunk], V moving [t, 64+1];
            out [s,65] accumulates over t in 4-chain PSUM banks; col 64 is
            the softmax denominator (ones column of vaug).
  norm    : per-partition reciprocal + tensor_scalar mult -> O bf16 [s, hd],
            head pairs packed [128 s, 128 hd], DMA-transposed into ot blocks.
  out proj: bf16 matmuls, ot stationary [hd, s-tile], W_out moving -> out
            [s, e] bf16, DMA'd per s-tile.
"""

import numpy as np
import ml_dtypes

import concourse.bass as bass
import concourse.tile as tile
from concourse import bacc, mybir
from concourse.alu_op_type import AluOpType
from concourse.bass_utils import run_bass_kernel_spmd

F32 = mybir.dt.float32
F16 = mybir.dt.float16
BF16 = mybir.dt.bfloat16
FP8 = mybir.dt.float8e4
I16 = mybir.dt.int16
EXP = mybir.ActivationFunctionType.Exp
IDENT = mybir.ActivationFunctionType.Identity
COPY = mybir.ActivationFunctionType.Copy
DR = mybir.MatmulPerfMode.DoubleRow

E = 1024          # embed dim
S = 2048          # sequence
B = 4             # batch
NH = 16           # total heads
HD = 64           # head dim
TP = 2            # head-group shards
HPC = NH // TP    # heads per core = 8
VW = HPC * HD     # 512 v columns per core
KCH = E // 128    # 8 contraction chunks

DITH = 2.0 ** -5                      # dither half-width
SP_, SM_ = 4.0 * (1 + DITH), 4.0 * (1 - DITH)
ESC = 1.0 / (256.0 * (1.0 + DITH * DITH))   # exp input scale (incl 1/sqrt(64))
SC1 = 1024.0 / np.log(2.0)            # fp16 schraudolph mult
SC2 = 15.0 * 1024.0 - 44.0            # fp16 schraudolph offset (tuned)
# exp engine per t-tile index: True -> DVE schraudolph, False -> ACT exact.
# Early units have PE-filler so ACT leads; late units split closer to even.
EXPMAP_EARLY = [False, False, True, False, False, True, False, True] * 2
EXPMAP_LATE = [False, True, False, True, False, True, False, True] * 2

_CACHE = {}


def build_nc():
    nc = bacc.Bacc("TRN2", target_bir_lowering=False, debug=False,
                   num_devices=8)

    xb_d = nc.dram_tensor("xb", [128, KCH, S], BF16, kind="ExternalInput").ap()
    wqk_d = nc.dram_tensor("wqk", [128, 8, KCH, 128], BF16,
                           kind="ExternalInput").ap()
    bqkp_d = nc.dram_tensor("bqkp", [128, 8], F32, kind="ExternalInput").ap()
    bqkm_d = nc.dram_tensor("bqkm", [128, 8], F32, kind="ExternalInput").ap()
    wv_d = nc.dram_tensor("wv", [128, KCH, VW], BF16,
                          kind="ExternalInput").ap()
    bv_d = nc.dram_tensor("bv", [128, 4], F32, kind="ExternalInput").ap()
    wo_d = nc.dram_tensor("wo", [128, 4, E], BF16, kind="ExternalInput").ap()
    out_d = nc.dram_tensor("out", [S, E], BF16, kind="ExternalOutput").ap()

    with tile.TileContext(nc) as tc:
        with (tc.tile_pool(name="persist", bufs=1) as pp,
              tc.tile_pool(name="bps", bufs=2, space="PSUM") as bps,
              tc.tile_pool(name="scps", bufs=2, space="PSUM") as scps,
              tc.tile_pool(name="oaps", bufs=2, space="PSUM") as oaps,
              tc.tile_pool(name="etp", bufs=20) as etp,
              tc.tile_pool(name="vst", bufs=2) as vst,
              tc.tile_pool(name="opr", bufs=2) as opr,
              tc.tile_pool(name="rcpp", bufs=4) as rcpp,
              tc.tile_pool(name="outp", bufs=3) as outp):
            xb = pp.tile([128, KCH, S], BF16, tag="xb")
            wqk = pp.tile([128, 8, KCH, 128], BF16, tag="wqk")
            bqkp = pp.tile([128, 8], F32, tag="bqkp")
            bqkm = pp.tile([128, 8], F32, tag="bqkm")
            wv = pp.tile([128, KCH, VW], BF16, tag="wv")
            bv = pp.tile([128, 4], F32, tag="bv")
            wo = pp.tile([128, 4, E], BF16, tag="wo")
            vaug = pp.tile([128, 16, HPC, 65], F16, tag="vaug")
            kt = [pp.tile([128, 2, S], FP8, tag=f"kt{i}", name=f"kt{i}")
                  for i in range(4)]
            qt = [pp.tile([128, 2, S], FP8, tag=f"qt{i}", name=f"qt{i}")
                  for i in range(4)]
            ot = [pp.tile([128, S], BF16, tag=f"ot{i}", name=f"ot{i}")
                  for i in range(4)]

            nc.vector.memset(vaug[:, :, :, 64:65], 1.0)

            # ---- input DMAs (order matters: m=4, m=0, wv/x early) ----
            morder = [4, 0, 5, 1, 6, 2, 7, 3]
            for k in range(KCH):
                eng = nc.sync if k % 2 == 0 else nc.scalar
                eng.dma_start(xb[:, k, :], xb_d[:, k, :])
            nc.sync.dma_start(wqk[:, 4, :, :], wqk_d[:, 4, :, :])
            nc.scalar.dma_start(wqk[:, 0, :, :], wqk_d[:, 0, :, :])
            nc.sync.dma_start(bqkp[:], bqkp_d[:])
            nc.sync.dma_start(bqkm[:], bqkm_d[:])
            nc.scalar.dma_start(bv[:], bv_d[:])
            nc.scalar.dma_start(wv[:], wv_d[:])
            for m in morder[2:]:
                eng = nc.sync if m % 2 == 0 else nc.scalar
                eng.dma_start(wqk[:, m, :, :], wqk_d[:, m, :, :])
            nc.scalar.dma_start(wo[:], wo_d[:])

            # ---- qk projection: m-tile -> dither pair in kt/qt ----
            def emit_qk(m, jlist=None):
                dest = kt[m - 4] if m >= 4 else qt[m]
                for j4 in jlist if jlist is not None else range(4):
                    ps = bps.tile([128, 512], F32, tag="ps",
                                  name=f"ps{m}_{j4}")
                    for k in range(KCH):
                        nc.tensor.matmul(
                            ps[:], wqk[:, m, k, :],
                            xb[:, k, j4 * 512:(j4 + 1) * 512],
                            start=(k == 0), stop=(k == KCH - 1))
                    sl = dest[:, 0, j4 * 512:(j4 + 1) * 512]
                    nc.vector.tensor_scalar(sl, ps[:], SP_, bqkp[:, m:m + 1],
                                            op0=AluOpType.mult,
                                            op1=AluOpType.add)
                    sl = dest[:, 1, j4 * 512:(j4 + 1) * 512]
                    nc.scalar.activation(sl, ps[:], IDENT,
                                         bias=bqkm[:, m:m + 1], scale=SM_)

            # ---- v projection (transposed layout), one vt = 2 heads ----
            def emit_v_quarter(vt, tc4):
                vsb = vst.tile([128, 512], F16, tag="vsb",
                               name=f"vsb{vt}_{tc4}")
                vp = bps.tile([128, 512], F32, tag="ps",
                              name=f"vp{vt}_{tc4}")
                for k in range(KCH):
                    nc.tensor.matmul(
                        vp[:], wv[:, k, vt * 128:(vt + 1) * 128],
                        xb[:, k, tc4 * 512:(tc4 + 1) * 512],
                        start=(k == 0), stop=(k == KCH - 1))
                nc.vector.tensor_scalar(
                    vsb[:], vp[:], bv[:, vt:vt + 1], None,
                    op0=AluOpType.add)
                sc4 = vst.tile([128, 4, 128], F16, tag="sc4",
                               name=f"sc4_{vt}_{tc4}")
                nc.sync.dma_start_transpose(sc4[:], vsb[:])
                nc.vector.tensor_copy(
                    vaug[:, tc4 * 4:(tc4 + 1) * 4, 2 * vt:2 * vt + 2, 0:64],
                    sc4[:].rearrange("p a (b c) -> p a b c", b=2))

            def emit_v(vt):
                for tc4 in range(4):
                    emit_v_quarter(vt, tc4)

            # preamble: minimal prefix for unit (h0, jj0) wave-0; the rest
            # drip-feeds as fine-grained filler popped inside attention units
            emit_qk(4)
            emit_qk(0, [0, 1])
            emit_v_quarter(0, 0)
            emit_v_quarter(0, 1)
            filler = [lambda: emit_v_quarter(0, 2), lambda: emit_v_quarter(0, 3),
                      lambda: emit_qk(0, [2]), lambda: emit_qk(0, [3])]
            for m, vt in [(5, None), (1, None), (None, 1), (6, None),
                          (2, None), (None, 2), (7, None), (3, None),
                          (None, 3)]:
                if m is not None:
                    filler += [lambda m=m, j=j: emit_qk(m, [j])
                               for j in range(4)]
                else:
                    filler += [lambda v=vt, t=t: emit_v_quarter(v, t)
                               for t in range(4)]

            def pop_filler():
                if filler:
                    filler.pop(0)()

            # ---- attention unit (h, jj): jj = s-half (1024 wide).
            # wave-0 V-matmuls interleave with wave-1 scores; wave-1
            # V-matmuls + per-st drain become "pending" closures popped
            # one per slot inside the NEXT unit's first scores loop, so
            # the exp engines never starve across unit boundaries.
            opair = {}

            def attn_unit(h, jj, pending):
                hp, p0 = h // 2, (h % 2) * 64
                uidx = h * 2 + jj
                expmap = EXPMAP_EARLY if uidx < 9 else EXPMAP_LATE
                oa = [oaps.tile([128, 4, 65], F32, tag="oa",
                                name=f"oa{h}_{jj}_{g}") for g in range(2)]
                ets = [None] * 16

                def scores_one(ti):
                    sc = scps.tile([128, 1024], F32, tag="sc",
                                   name=f"sc{h}_{jj}_{ti}")
                    for sh in range(2):
                        nc.tensor.matmul(
                            sc[:, sh * 512:(sh + 1) * 512],
                            kt[hp][p0:p0 + 64, :, ti * 128:(ti + 1) * 128],
                            qt[hp][p0:p0 + 64, :,
                                   jj * 1024 + sh * 512:
                                   jj * 1024 + (sh + 1) * 512],
                            start=True, stop=True, perf_mode=DR)
                    et = etp.tile([128, 1024], F16, tag="et",
                                  name=f"et{h}_{jj}_{ti}")
                    if expmap[ti]:
                        nc.vector.tensor_scalar(
                            et[:].bitcast(I16), sc[:], ESC * SC1, SC2,
                            op0=AluOpType.mult, op1=AluOpType.add)
                    else:
                        nc.scalar.activation(et[:], sc[:], EXP, scale=ESC)
                    ets[ti] = et

                def vmm(st, ti):
                    nc.tensor.matmul(
                        oa[st // 4][:, st % 4, :],
                        ets[ti][:, st * 128:(st + 1) * 128],
                        vaug[:, ti, h, :],
                        start=(ti == 0), stop=(ti == 15))

                for tl in range(8):
                    scores_one(tl)
                    if pending:
                        pending.pop(0)()
                    if tl % 4 == 3:
                        pop_filler()
                for tl in range(8):
                    scores_one(8 + tl)
                    for t0 in range(8):
                        vmm(tl, t0)
                    if tl == 3 or (tl == 7 and uidx < 4):
                        pop_filler()

                if (hp, jj) not in opair:
                    opair[(hp, jj)] = opr.tile([128, 8, 128], BF16, tag="op",
                                               name=f"op{hp}_{jj}")
                osb = opair[(hp, jj)]

                def piece(st):
                    for tl in range(8):
                        vmm(st, 8 + tl)
                    rcp = rcpp.tile([128, 1], F32, tag="rcp")
                    nc.vector.reciprocal(rcp[:], oa[st // 4][:, st % 4,
                                                             64:65])
                    dst = osb[:, st, (h % 2) * 64:(h % 2) * 64 + 64]
                    src = oa[st // 4][:, st % 4, 0:64]
                    if st % 2 == 0:
                        nc.vector.tensor_scalar(dst, src, rcp[:], None,
                                                op0=AluOpType.mult)
                    else:
                        nc.scalar.activation(dst, src, COPY, scale=rcp[:])
                    if h % 2 == 1 and st == 7:
                        nc.sync.dma_start_transpose(
                            ot[hp][:, jj * 1024:(jj + 1) * 1024].rearrange(
                                "p (a c) -> p a c", c=128),
                            osb[:].rearrange("p a c -> p (a c)"))
                        del opair[(hp, jj)]

                return [lambda st=st: piece(st) for st in range(8)]

            # ---- out projection for one s-tile ----
            def emit_out(st):
                osb = outp.tile([128, E], BF16, tag="outsb", name=f"os{st}")
                for e2 in range(2):
                    op = bps.tile([128, 512], F32, tag="ps",
                                  name=f"op{st}_{e2}")
                    for kc in range(4):
                        nc.tensor.matmul(
                            op[:], ot[kc][:, st * 128:(st + 1) * 128],
                            wo[:, kc, e2 * 512:(e2 + 1) * 512],
                            start=(kc == 0), stop=(kc == 3))
                    if e2 == 0:
                        nc.scalar.activation(
                            osb[:, e2 * 512:(e2 + 1) * 512], op[:], COPY)
                    else:
                        nc.vector.tensor_copy(
                            osb[:, e2 * 512:(e2 + 1) * 512], op[:])
                nc.gpsimd.dma_start(out_d[st * 128:(st + 1) * 128, :],
                                    osb[:])

            # ---- main schedule: h-outer, projections drip-fed as filler,
            # per-unit trailing work software-pipelined via `pending`.
            pending = []
            for h in range(HPC):
                for jj in range(2):
                    pending = attn_unit(h, jj, pending)
            while filler:
                filler.pop(0)()
            # u15=(h7,jj1) popped u14's pending, so jj0 s-tiles are ready
            for st in range(8):
                emit_out(st)
            while pending:
                pending.pop(0)()
            for st in range(8, 16):
                emit_out(st)

    nc.compile()
    return nc


def _shard_inputs(x, W_qkv, b_qkv, W_out, b_out):
    BF = ml_dtypes.bfloat16
    xbs = []
    for b in range(B):
        xT = np.ascontiguousarray(x[b].T)                       # [E, S]
        xbs.append(np.ascontiguousarray(
            xT.reshape(KCH, 128, S).transpose(1, 0, 2)).astype(BF))
    gshards = []
    for g in range(TP):
        lo, hi = g * VW, (g + 1) * VW
        Wq = W_qkv[:, lo:hi]
        Wk = W_qkv[:, E + lo:E + hi]
        Wv_ = W_qkv[:, 2 * E + lo:2 * E + hi]
        bq = b_qkv[lo:hi]
        bk = b_qkv[E + lo:E + hi]
        bvv = b_qkv[2 * E + lo:2 * E + hi]
        Wqk = np.concatenate([Wq, Wk], axis=1)                  # [E, 1024]
        wqk = np.ascontiguousarray(
            Wqk.reshape(KCH, 128, 8, 128).transpose(1, 2, 0, 3)).astype(BF)
        bcat = np.concatenate([bq, bk]).reshape(8, 128).T       # [128, 8]
        wv = np.ascontiguousarray(
            Wv_.reshape(KCH, 128, VW).transpose(1, 0, 2)).astype(BF)
        bvt = bvv.reshape(4, 128).T                             # [128, 4]
        wo = np.ascontiguousarray(
            W_out[lo:hi, :].reshape(4, 128, E).transpose(1, 0, 2)).astype(BF)
        gshards.append({
            "wqk": wqk,
            "bqkp": np.ascontiguousarray(bcat * SP_, dtype=np.float32),
            "bqkm": np.ascontiguousarray(bcat * SM_, dtype=np.float32),
            "wv": wv,
            "bv": np.ascontiguousarray(bvt, dtype=np.float32),
            "wo": wo,
        })
    in_maps = []
    for c in range(8):
        b, g = c // TP, c % TP
        m = dict(gshards[g])
        m["xb"] = xbs[b]
        in_maps.append(m)
    return in_maps


def kernel(x, W_qkv, b_qkv, W_out, b_out):
    x = np.asarray(x, dtype=np.float32)
    W_qkv = np.asarray(W_qkv, dtype=np.float32)
    b_qkv = np.asarray(b_qkv, dtype=np.float32)
    W_out = np.asarray(W_out, dtype=np.float32)
    b_out = np.asarray(b_out, dtype=np.float32)
    if "nc" not in _CACHE:
        _CACHE["nc"] = build_nc()
    nc = _CACHE["nc"]
    in_maps = _shard_inputs(x, W_qkv, b_qkv, W_out, b_out)
    res = None
    for attempt in range(3):
        try:
            res = run_bass_kernel_spmd(nc, in_maps, core_ids=list(range(8)))
            break
        except Exception:
            if attempt == 2:
                raise
    _CACHE["last_results"] = res
    out = np.empty((B, S, E), dtype=np.float32)
    for b in range(B):
        out[b] = (res.results[TP * b]["out"].astype(np.float32) +
                  res.results[TP * b + 1]["out"].astype(np.float32) + b_out)
    return out
